# revision 7
# baseline (speedup 1.0000x reference)
"""CoefficientMaxPool Trainium2 kernel (8-core data-parallel), v3.

Problem: x [32, 512, 16, 128] f32.  Irreps group into degree blocks
l=0:[0,1), l=1:[1,4), l=2:[4,9), l=3:[9,16).  Per (batch, l, channel):
find the neighbor n* maximizing the degree-block squared norm, output
that neighbor's block components -> out [32, 16, 128].

Per core (4 batches), per batch:
  - DMA x[b] as [p=128, a=4, i=16, c=128] (n = a*128 + p), per-a chunks
  - ACT: x2 = x*x (per-half)
  - PE transpose-accumulate x2 i-planes -> NT_l [c, n] in PSUM (fp32,
    exact argmax; the i-sum within a degree block rides the PSUM
    accumulate of the transpose matmuls)
  - DVE: maxv_l[c] = reduce-max over n; maskT_l[c,n] = (NT_l == maxv_l)
    emitted as bf16 (0/1 exact)
  - PE: transpose bf16 masks back -> MP_l [n, (a c)] PSUM (1 cyc/row)
  - winner-select Xb = X * MP[l(i)] -> bf16: l2+l1 on DVE (PSUM masks),
    l3+l0 on GPSIMD (needs SBUF masks -> ACT copies those two)
  - PE: ones^T @ Xb (bf16: 1 cyc/row) accumulated over a -> out halves
    [1, 2, 512] fp32 in PSUM; ACT copy -> SBUF; DMA out.

The final sum + out copy for batch b are emitted one loop iteration
late (software pipelining): the PE fills the winner-select wait with
batch b+1's norm transposes, staying continuously busy (which also
keeps its clock at full speed).
"""

import os
import sys

import numpy as np

for _p in ("/opt/trn_rl_repo", "/opt/pypackages"):
    if _p not in sys.path:
        sys.path.append(_p)

from contextlib import ExitStack

import concourse.bacc as bacc
import concourse.bass as bass
import concourse.tile as tile
from concourse import mybir

N_CORES = 8
B_FULL, N, IRR, C = 32, 512, 16, 128
B = B_FULL // N_CORES  # 4 batches per core
P = 128                # partitions (n within chunk)
A = N // P             # 4 neighbor chunks
BLOCKS = [(0, 1), (1, 4), (4, 9), (9, 16)]  # irrep ranges per degree l
F32 = mybir.dt.float32
BF16 = mybir.dt.bfloat16
MAX = mybir.AluOpType.max
MULT = mybir.AluOpType.mult
EQ = mybir.AluOpType.is_equal

# priority order: l2 first (gates DVE winner-mult), l3 (gates GPSIMD),
# then l1, l0
L_ORDER = (2, 3, 1, 0)

_cache = {}


def _build_bass():
    nc = bacc.Bacc("TRN2", target_bir_lowering=False, debug=False,
                   num_devices=N_CORES)
    x_in = nc.dram_tensor("x", [B, N, IRR, C], F32, kind="ExternalInput")
    out_t = nc.dram_tensor("out", [B, IRR, C], F32, kind="ExternalOutput")
    ident_d = nc.inline_tensor(np.eye(P, dtype=np.float32), name="ident")

    with tile.TileContext(nc) as tc, ExitStack() as ctx:
        # DRAM view: n = a*P + p  ->  [b, p, a, i, c]
        x_v = x_in.ap().rearrange("b (a p) i c -> b p a i c", p=P)
        out_v = out_t.ap().rearrange("b i c -> b (i c)")

        xp = ctx.enter_context(tc.tile_pool(name="xp", bufs=2))
        x2p = ctx.enter_context(tc.tile_pool(name="x2p", bufs=2))
        xbp = ctx.enter_context(tc.tile_pool(name="xbp", bufs=2))
        mtp = ctx.enter_context(tc.tile_pool(name="mtp", bufs=2))
        msp = ctx.enter_context(tc.tile_pool(name="msp", bufs=2))
        mvp = ctx.enter_context(tc.tile_pool(name="mvp", bufs=2))
        obp = ctx.enter_context(tc.tile_pool(name="obp", bufs=2))
        singles = ctx.enter_context(tc.tile_pool(name="singles", bufs=1))
        psc = ctx.enter_context(tc.tile_pool(name="psc", bufs=4, space="PSUM"))
        pout = ctx.enter_context(tc.tile_pool(name="pout", bufs=2, space="PSUM"))

        ones_b = singles.tile([P, 1], BF16)
        nc.vector.memset(ones_b, 1.0)
        ident = singles.tile([P, P], F32)
        nc.sync.dma_start(out=ident, in_=ident_d.ap())
        ident_b = singles.tile([P, P], BF16)
        nc.scalar.copy(out=ident_b, in_=ident)

        Xs, Xbs = {}, {}

        def emit_sum_and_out(b):
            # batch b's masked sum over n + out copy + store (pipelined late)
            Xf = Xbs[b].rearrange("p a i c -> p a (i c)")
            ob = obp.tile([1, IRR * C], F32, tag="ob", name="ob")
            for h in range(2):
                po = pout.tile([1, 2, 512], F32, tag="po", name="po")
                for kk in range(2):
                    k = 2 * h + kk
                    for a in range(A):
                        nc.tensor.matmul(po[:, kk, :], ones_b,
                                         Xf[:, a, k * 512:(k + 1) * 512],
                                         start=(a == 0), stop=(a == A - 1))
                nc.scalar.copy(out=ob[:, h * 1024:(h + 1) * 1024],
                               in_=po.rearrange("m k f -> m (k f)"))
            nc.sync.dma_start(out=out_v[b].unsqueeze(0), in_=ob)

        for b in range(B + 1):
            if b < B:
                X = xp.tile([P, A, IRR, C], F32, tag="X", name="X")
                X2 = x2p.tile([P, A, IRR, C], F32, tag="X2", name="X2")
                Xb = xbp.tile([P, A, IRR, C], BF16, tag="Xb", name="Xb")
                Xs[b], Xbs[b] = X, Xb
                for a in range(A):
                    nc.sync.dma_start(out=X[:, a], in_=x_v[b][:, a])
                for h in range(2):
                    ha = slice(2 * h, 2 * h + 2)
                    nc.scalar.activation(X2[:, ha], X[:, ha],
                                         mybir.ActivationFunctionType.Square)

                # transposed degree norms NT_l [c, n=a*128+p] in PSUM (fp32)
                NT = {}
                for l in L_ORDER:
                    NT[l] = psc.tile([P, A * P], F32, tag="sc", name=f"NT{l}")
                for a in range(A):
                    sl = slice(a * P, (a + 1) * P)
                    for l in L_ORDER:
                        s, e = BLOCKS[l]
                        for i in range(s, e):
                            nc.tensor.matmul(NT[l][:, sl], X2[:, a, i, :],
                                             ident, is_transpose=True,
                                             start=(i == s), stop=(i == e - 1))

            if b > 0:
                emit_sum_and_out(b - 1)

            if b < B:
                # per-l: max over n, equality mask (transposed, SBUF, bf16)
                maskT = {}
                for l in L_ORDER:
                    maxv = mvp.tile([P, 1], F32, tag=f"maxv{l}",
                                    name=f"maxv{l}")
                    nc.vector.tensor_reduce(
                        out=maxv, in_=NT[l], axis=mybir.AxisListType.X, op=MAX)
                    mt = mtp.tile([P, A * P], BF16, tag=f"mt{l}",
                                  name=f"mt{l}")
                    nc.vector.tensor_scalar(
                        out=mt, in0=NT[l], scalar1=maxv, scalar2=None, op0=EQ)
                    maskT[l] = mt

                # transpose bf16 masks back -> MP_l [p, (a c)] PSUM
                MP = {}
                for l in L_ORDER:
                    mp = psc.tile([P, A * P], BF16, tag="sc", name=f"MP{l}")
                    for a in range(A):
                        sl = slice(a * P, (a + 1) * P)
                        nc.tensor.matmul(mp[:, sl], maskT[l][:, sl], ident_b,
                                         is_transpose=True)
                    MP[l] = mp

                # SBUF copies of l3/l0 masks for GPSIMD (no PSUM access)
                MS = {}
                for l in (3, 0):
                    ms = msp.tile([P, A, C], BF16, tag=f"ms{l}",
                                  name=f"ms{l}")
                    nc.scalar.copy(out=ms,
                                   in_=MP[l].rearrange("p (a c) -> p a c",
                                                       a=A))
                    MS[l] = ms

                # winner-select: Xb = X * MP[l(i)]  (bf16 out)
                def wm(eng, l, mask):
                    s, e = BLOCKS[l]
                    eng.tensor_tensor(
                        Xb[:, :, s:e, :], X[:, :, s:e, :],
                        mask.unsqueeze(2).broadcast_to([P, A, e - s, C]),
                        MULT)

                wm(nc.vector, 2, MP[2].rearrange("p (a c) -> p a c", a=A))
                wm(nc.gpsimd, 3, MS[3])
                wm(nc.vector, 1, MP[1].rearrange("p (a c) -> p a c", a=A))
                wm(nc.gpsimd, 0, MS[0])

    nc.compile()
    return nc


def kernel(x: np.ndarray, i2l: np.ndarray | None = None) -> np.ndarray:
    x = np.ascontiguousarray(np.asarray(x), dtype=np.float32)
    assert x.shape == (B_FULL, N, IRR, C), x.shape

    if "nc" not in _cache:
        _cache["nc"] = _build_bass()
    nc = _cache["nc"]

    from concourse.bass_utils import run_bass_kernel_spmd

    in_maps = [{"x": x[i * B:(i + 1) * B]} for i in range(N_CORES)]
    res = run_bass_kernel_spmd(nc, in_maps, list(range(N_CORES)))
    out = np.concatenate([res.results[i]["out"] for i in range(N_CORES)], axis=0)
    return out


if __name__ == "__main__":
    xs = np.random.randn(B_FULL, N, IRR, C).astype(np.float32)
    o = kernel(xs)
    print("out", o.shape, o.dtype)
